# revision 5
# baseline (speedup 1.0000x reference)
"""TRN2 Bass kernel for nn_BeyazKusAIAttention_36515811951168.

Key reduction: the reference applies softmax over a size-1 axis, which is
identically 1.0, so attention weights are exactly 1 and the module collapses
to
    y = (x @ Wv^T) @ Wfold^T,  with  Wfold = Wo.reshape(4096,4,1024).sum(1)
(q/rope/scores/mask are dead code; `out` is v tiled over the 4 heads, and the
o-projection of the tiled v folds head-wise into Wfold).  This is a 5x FLOP
reduction vs the reference graph.

Execution: data-parallel over the 16384 = batch*seq rows across 8 NeuronCores
(no collectives).  Both matmuls run in fp16 (1 PE cycle/row, fp32 PSUM
accumulation); measured end-to-end relative error vs the fp32 reference is
~3e-4.  Per-core PE floor is 2*2048*(4096*1024+1024*4096) FLOP / 78.6 TF
= 437 us; everything else is structured to keep the PE stream gapless:

  - Wv^T (8 MB) and Wfold^T (8 MB) are SBUF-resident, loaded once per
    launch; only x^T streams in (16 MB) and y streams out (32 MB).
  - Per 512-row chunk: MM1 k-major (each x^T k-slice feeds 8 matmuls into
    8 PSUM banks, K=4096 accumulated), DVE evicts v^T to SBUF fp16, MM2
    (v^T stationary, Wfold^T moving, K=1024) n-major so Wfold tiles are
    needed progressively during the first chunk while they stream in.
  - PSUM tags rotate so MM2's bank n*4+sub reuses MM1's bank just after
    its eviction, and MM1 of the next chunk reuses MM2's banks with
    ~8 matmuls of slack.

Host-side layouts (partition dim = contraction dim for both matmuls):
  xt [32,128,R]: xt[k,p,r] = x[row r, dim 128k+p]     (transposed shard)
  wvt[32,128,1024]: wvt[k,p,m] = Wv[m, 128k+p]
  wft[8,128,4096]:  wft[k,p,n] = Wfold[n, 128k+p]
  y  [R/128,128,4096]: y[t,p,n] = out[row 128t+p, n]
"""
import numpy as np
import concourse.bass as bass
from concourse import bacc
import concourse.mybir as mybir
from concourse.tile import TileContext
from concourse.bass_utils import run_bass_kernel_spmd

DIM = 4096
KV = 1024
N_CORES = 8
ROWS_TOTAL = 4 * 4096
ROWS = ROWS_TOTAL // N_CORES   # 2048
KT1 = DIM // 128               # 32 k-tiles, phase 1
MT1 = KV // 128                # 8 vcol tiles
KT2 = KV // 128                # 8 k-tiles, phase 2
NC2 = DIM // 512               # 8 ycol chunks

_nc_cache = {}


class _nullctx:
    def __enter__(self):
        return None

    def __exit__(self, *a):
        return False


def _build(rows=ROWS, loop_n=1):
    """Fused single-pass program; see module docstring.

    loop_n > 1 wraps the whole per-core program in a hardware loop for
    slope timing (device time per iteration).
    """
    CH = 512
    nch = rows // CH
    f32 = mybir.dt.float32
    f16 = mybir.dt.float16

    nc = bacc.Bacc(None, target_bir_lowering=False)
    XT = nc.dram_tensor("xt", [KT1, 128, rows], f16, kind="ExternalInput")
    WVT = nc.dram_tensor("wvt", [KT1, 128, KV], f16, kind="ExternalInput")
    WFT = nc.dram_tensor("wft", [KT2, 128, DIM], f16, kind="ExternalInput")
    Y = nc.dram_tensor("y", [rows // 128, 128, DIM], f32,
                       kind="ExternalOutput")

    with TileContext(nc) as tc:
        with (
            tc.tile_pool(name="wv", bufs=1) as wvpool,
            tc.tile_pool(name="wf", bufs=1) as wfpool,
            tc.tile_pool(name="xts", bufs=8) as xtpool,
            tc.tile_pool(name="vss", bufs=2) as vspool,
            tc.tile_pool(name="yst", bufs=4) as ypool,
            tc.tile_pool(name="ps", bufs=1, space="PSUM") as pspool,
            tc.For_i(0, loop_n) if loop_n > 1 else _nullctx(),
        ):
            # resident weights: Wv^T first (gates MM1), Wfold^T streams in
            # during chunk 0 (n-th tile needed ~55+7n us into the launch)
            wv = []
            for k in range(KT1):
                wvk = wvpool.tile([128, KV], f16, tag=f"wv{k}")
                nc.sync.dma_start(wvk[:], WVT[k])
                wv.append(wvk)
            wf = []
            for n in range(NC2):
                wfn = wfpool.tile([128, KT2, 512], f16, tag=f"wf{n}")
                for k in range(KT2):
                    nc.sync.dma_start(wfn[:, k, :],
                                      WFT[k, :, n * 512:(n + 1) * 512])
                wf.append(wfn)
            for rc in range(nch):
                # MM1: v^T[m, rows] += wv[k]^T slices @ x^T[k]; k-major so
                # each streamed xt tile feeds 8 back-to-back matmuls
                ps1 = [pspool.tile([128, CH], f32, tag=f"ps{m}",
                                   name=f"ps1_{rc}_{m}")
                       for m in range(MT1)]
                for k in range(KT1):
                    xtk = xtpool.tile([128, CH], f16, tag="xt")
                    nc.sync.dma_start(xtk[:],
                                      XT[k, :, rc * CH:(rc + 1) * CH])
                    for m in range(MT1):
                        nc.tensor.matmul(
                            ps1[m][:], wv[k][:, m * 128:(m + 1) * 128],
                            xtk[:], start=(k == 0), stop=(k == KT1 - 1))
                vs = []
                for m in range(MT1):
                    v = vspool.tile([128, CH], f16, tag=f"vs{m}")
                    nc.vector.tensor_copy(v[:], ps1[m][:])
                    vs.append(v)
                # MM2: y[rows, n] += v^T slices (stationary) @ wf[n];
                # n-major so wf[n] is first needed ~6.8*n us into MM2
                for n in range(NC2):
                    for sub in range(CH // 128):
                        ps2 = pspool.tile(
                            [128, 512], f32,
                            tag=f"ps{(n * (CH // 128) + sub) % MT1}")
                        for k2 in range(KT2):
                            nc.tensor.matmul(
                                ps2[:],
                                vs[k2][:, sub * 128:(sub + 1) * 128],
                                wf[n][:, k2, :],
                                start=(k2 == 0), stop=(k2 == KT2 - 1))
                        ys = ypool.tile([128, 512], f32, tag="ys")
                        nc.vector.tensor_copy(ys[:], ps2[:])
                        nc.sync.dma_start(
                            Y[rc * (CH // 128) + sub, :,
                              n * 512:(n + 1) * 512], ys[:])
    nc.compile()
    return nc


def prep_in_maps(inputs):
    x = np.asarray(inputs["x"])
    Wv = np.asarray(inputs["Wv"], dtype=np.float32)
    Wo = np.asarray(inputs["Wo"], dtype=np.float32)

    # host-side relayout: transpose x once, fold Wo over heads
    x2 = np.ascontiguousarray(
        x.reshape(ROWS_TOTAL, DIM).T).astype(np.float16)
    xt_all = x2.reshape(KT1, 128, ROWS_TOTAL)
    wvt = np.ascontiguousarray(Wv.T).astype(np.float16).reshape(KT1, 128, KV)
    wfold = Wo.reshape(DIM, 4, KV).sum(axis=1)
    wft = np.ascontiguousarray(
        wfold.T).astype(np.float16).reshape(KT2, 128, DIM)

    in_maps = []
    for c in range(N_CORES):
        in_maps.append({
            "xt": np.ascontiguousarray(
                xt_all[:, :, c * ROWS:(c + 1) * ROWS]),
            "wvt": wvt,
            "wft": wft,
        })
    return in_maps


def get_nc():
    if "nc" not in _nc_cache:
        _nc_cache["nc"] = _build()
    return _nc_cache["nc"]


def kernel(x, Wq, Wk, Wv, Wo, mask):
    x = np.asarray(x)
    B, S, D = x.shape
    assert D == DIM and B * S == ROWS_TOTAL

    in_maps = prep_in_maps({"x": x, "Wv": Wv, "Wo": Wo})
    nc = get_nc()

    # transient NRT device errors (e.g. NRT_EXEC_UNIT_UNRECOVERABLE right
    # after another process released the cores) succeed on retry
    last_err = None
    for _attempt in range(3):
        try:
            results = run_bass_kernel_spmd(
                nc, in_maps, core_ids=list(range(N_CORES))).results
            break
        except Exception as e:  # noqa: BLE001
            last_err = e
    else:
        raise last_err
    shards = [r["y"].reshape(ROWS, DIM) for r in results]
    out = np.concatenate(shards, axis=0).reshape(B, S, DIM)
    return out.astype(np.float32, copy=False)
